# revision 1
# baseline (speedup 1.0000x reference)
"""LongTermMemory retrieval (cosine-sim KNN, top-16, softmax-weighted gather)
as a Bass/Tile kernel for 8 Trainium2 NeuronCores.

Strategy: data-parallel over the B*T=4096 queries (512 queries per core),
ltm_buffer replicated. Each core:
  - normalizes its queries and PE-transposes them to (D, q) layout
  - streams the memory buffer in 32 tiles of 512 rows: row-normalize,
    PE-transpose to (D, m) layout, fp32 matmul (exact scores needed: the
    smallest top-16/17 score gap in this data is ~2.5e-7)
  - keeps per-tile top-8 candidate score values (DVE max), spills full score
    rows to a DRAM scratch
  - per 128-query chunk: top-16 values from the 256 candidates, indices via
    max_index over the reloaded score row, softmax, 16 indirect row gathers
    of the un-normalized buffer, weighted sum.

All inputs/outputs are full (unsharded); sharding happens on the host here.
"""

import numpy as np

import concourse.bass as bass
import concourse.bacc as bacc
import concourse.tile as tile
import concourse.mybir as mybir
from concourse import bass_utils
from concourse.masks import make_identity

P = 128
B, T, D, M = 2, 2048, 1024, 16384
TOPK = 16
NCORES = 8
Q = B * T                  # 4096 queries total
QPC = Q // NCORES          # 512 queries per core
NQCH = QPC // P            # 4 query chunks of 128
MTILE = 512                # memory rows per tile
NMT = M // MTILE           # 32 memory tiles
NSUB = MTILE // P          # 4 row-subtiles per memory tile
KCH = D // P               # 8 contraction chunks
CAND = NMT * 8             # 256 candidate values per query

f32 = mybir.dt.float32
u32 = mybir.dt.uint32

_cache = {}


def _build():
    nc = bacc.Bacc("TRN2", target_bir_lowering=False, debug=False, num_devices=NCORES)

    xs_d = nc.dram_tensor("xs", (QPC, D), f32, kind="ExternalInput").ap()
    mem_d = nc.dram_tensor("mem", (M, D), f32, kind="ExternalInput").ap()
    out_d = nc.dram_tensor("out", (QPC, D), f32, kind="ExternalOutput").ap()
    scr_d = nc.dram_tensor("scr", (NQCH, P, M), f32, kind="Internal").ap()

    ACT = mybir.ActivationFunctionType
    OP = mybir.AluOpType

    with tile.TileContext(nc) as tc:
        with tc.tile_pool(name="persist", bufs=1) as pp:
            ident = pp.tile([P, P], f32)
            make_identity(nc, ident[:])
            qT = pp.tile([P, KCH, QPC], f32)       # (d_in_slice, k, q)
            cand = pp.tile([P, NQCH, CAND], f32)   # per-chunk candidate values

            # ---------------- Phase A: queries -> normalized, transposed ----
            with tc.tile_pool(name="pa", bufs=2) as pa, \
                 tc.tile_pool(name="pa_ps", bufs=2, space="PSUM") as paps:
                for c in range(NQCH):
                    xq = pa.tile([P, D], f32)
                    nc.sync.dma_start(out=xq[:], in_=xs_d[c * P:(c + 1) * P, :])
                    sq = pa.tile([P, D], f32)
                    ssq = pa.tile([P, 1], f32)
                    nc.scalar.activation(out=sq[:], in_=xq[:], func=ACT.Square,
                                         accum_out=ssq[:])
                    nrm = pa.tile([P, 1], f32)
                    nc.scalar.activation(out=nrm[:], in_=ssq[:], func=ACT.Sqrt)
                    rn = pa.tile([P, 1], f32)
                    nc.vector.reciprocal(out=rn[:], in_=nrm[:])
                    qn = pa.tile([P, D], f32)
                    nc.vector.tensor_scalar(out=qn[:], in0=xq[:],
                                            scalar1=rn[:, :1], scalar2=None,
                                            op0=OP.mult)
                    for kh in range(2):
                        tp = paps.tile([P, 4 * P], f32, space="PSUM")
                        for i in range(4):
                            k = kh * 4 + i
                            nc.tensor.transpose(out=tp[:, i * P:(i + 1) * P],
                                                in_=qn[:, k * P:(k + 1) * P],
                                                identity=ident[:])
                        nc.scalar.copy(
                            out=qT[:, kh * 4:(kh + 1) * 4, c * P:(c + 1) * P],
                            in_=tp[:].rearrange("p (i j) -> p i j", i=4))

            # ---------------- Phase B: score all memory tiles ---------------
            with tc.tile_pool(name="pb", bufs=2) as pb, \
                 tc.tile_pool(name="pb_sc", bufs=4) as pbs, \
                 tc.tile_pool(name="pb_ps", bufs=2, space="PSUM") as pbps, \
                 tc.tile_pool(name="pb_mm", bufs=3, space="PSUM") as pbmm:
                for mt in range(NMT):
                    memr = pb.tile([P, NSUB, D], f32)
                    nc.sync.dma_start(
                        out=memr[:],
                        in_=mem_d[mt * MTILE:(mt + 1) * MTILE, :]
                        .rearrange("(s p) d -> p s d", p=P))
                    ssq4 = pb.tile([P, NSUB], f32)
                    sq = pb.tile([P, D], f32)
                    for s in range(NSUB):
                        nc.scalar.activation(out=sq[:], in_=memr[:, s, :],
                                             func=ACT.Square,
                                             accum_out=ssq4[:, s:s + 1])
                    nrm4 = pb.tile([P, NSUB], f32)
                    nc.scalar.activation(out=nrm4[:], in_=ssq4[:], func=ACT.Sqrt)
                    rn4 = pb.tile([P, NSUB], f32)
                    nc.vector.reciprocal(out=rn4[:], in_=nrm4[:])
                    for s in range(NSUB):
                        nc.vector.tensor_scalar(out=memr[:, s, :],
                                                in0=memr[:, s, :],
                                                scalar1=rn4[:, s:s + 1],
                                                scalar2=None, op0=OP.mult)
                    memT = pb.tile([P, KCH, MTILE], f32)
                    for s in range(NSUB):
                        for kh in range(2):
                            tp = pbps.tile([P, 4 * P], f32, space="PSUM")
                            for i in range(4):
                                k = kh * 4 + i
                                nc.tensor.transpose(
                                    out=tp[:, i * P:(i + 1) * P],
                                    in_=memr[:, s, k * P:(k + 1) * P],
                                    identity=ident[:])
                            nc.scalar.copy(
                                out=memT[:, kh * 4:(kh + 1) * 4, s * P:(s + 1) * P],
                                in_=tp[:].rearrange("p (i j) -> p i j", i=4))
                    for c in range(NQCH):
                        ps = pbmm.tile([P, MTILE], f32, space="PSUM")
                        for k in range(KCH):
                            nc.tensor.matmul(out=ps[:],
                                             lhsT=qT[:, k, c * P:(c + 1) * P],
                                             rhs=memT[:, k, :],
                                             start=(k == 0), stop=(k == KCH - 1))
                        sc = pbs.tile([P, MTILE], f32)
                        nc.vector.tensor_copy(out=sc[:], in_=ps[:])
                        nc.vector.max(out=cand[:, c, mt * 8:(mt + 1) * 8],
                                      in_=sc[:])
                        nc.sync.dma_start(
                            out=scr_d[c, :, mt * MTILE:(mt + 1) * MTILE],
                            in_=sc[:])

            # ---------------- Phase C: select, softmax, gather, combine -----
            with tc.tile_pool(name="pc_row", bufs=1) as pcr, \
                 tc.tile_pool(name="pc", bufs=2) as pc, \
                 tc.tile_pool(name="pc_g", bufs=4) as pcg:
                for c in range(NQCH):
                    srow = pcr.tile([P, M], f32)
                    nc.sync.dma_start(out=srow[:], in_=scr_d[c])
                    vals16 = pc.tile([P, TOPK], f32)
                    nc.vector.max(out=vals16[:, 0:8], in_=cand[:, c, :])
                    crep = pc.tile([P, CAND], f32)
                    nc.vector.match_replace(out=crep[:],
                                            in_to_replace=vals16[:, 0:8],
                                            in_values=cand[:, c, :],
                                            imm_value=-1e30)
                    nc.vector.max(out=vals16[:, 8:16], in_=crep[:])
                    idx = pc.tile([P, TOPK], u32)
                    nc.vector.max_index(out=idx[:, 0:8], in_max=vals16[:, 0:8],
                                        in_values=srow[:])
                    nc.vector.max_index(out=idx[:, 8:16], in_max=vals16[:, 8:16],
                                        in_values=srow[:])
                    # softmax over the 16 values (order-invariant)
                    nvmax = pc.tile([P, 1], f32)
                    nc.vector.tensor_scalar(out=nvmax[:], in0=vals16[:, 0:1],
                                            scalar1=-1.0, scalar2=None,
                                            op0=OP.mult)
                    ex16 = pc.tile([P, TOPK], f32)
                    esum = pc.tile([P, 1], f32)
                    nc.scalar.activation(out=ex16[:], in_=vals16[:], func=ACT.Exp,
                                         bias=nvmax[:, :1], scale=1.0,
                                         accum_out=esum[:])
                    rsum = pc.tile([P, 1], f32)
                    nc.vector.reciprocal(out=rsum[:], in_=esum[:])
                    w16 = pc.tile([P, TOPK], f32)
                    nc.vector.tensor_scalar(out=w16[:], in0=ex16[:],
                                            scalar1=rsum[:, :1], scalar2=None,
                                            op0=OP.mult)
                    acc = pc.tile([P, D], f32)
                    for j in range(TOPK):
                        g = pcg.tile([P, D], f32)
                        nc.gpsimd.indirect_dma_start(
                            out=g[:], out_offset=None, in_=mem_d[:],
                            in_offset=bass.IndirectOffsetOnAxis(
                                ap=idx[:, j:j + 1], axis=0))
                        if j == 0:
                            nc.scalar.activation(out=acc[:], in_=g[:],
                                                 func=ACT.Copy,
                                                 scale=w16[:, j:j + 1])
                        else:
                            gs = pcg.tile([P, D], f32)
                            nc.scalar.activation(out=gs[:], in_=g[:],
                                                 func=ACT.Copy,
                                                 scale=w16[:, j:j + 1])
                            nc.vector.tensor_tensor(out=acc[:], in0=acc[:],
                                                    in1=gs[:], op=OP.add)
                    nc.sync.dma_start(out=out_d[c * P:(c + 1) * P, :], in_=acc[:])

    nc.compile()
    return nc


def kernel(x, ltm_buffer, top_k):
    assert int(top_k) == TOPK
    x = np.ascontiguousarray(np.asarray(x, dtype=np.float32)).reshape(Q, D)
    ltm = np.ascontiguousarray(np.asarray(ltm_buffer, dtype=np.float32))

    if "nc" not in _cache:
        _cache["nc"] = _build()
    nc = _cache["nc"]

    in_maps = [
        {"xs": x[i * QPC:(i + 1) * QPC], "mem": ltm}
        for i in range(NCORES)
    ]
    res = bass_utils.run_bass_kernel_spmd(nc, in_maps, core_ids=list(range(NCORES)))
    out = np.concatenate([res.results[i]["out"] for i in range(NCORES)], axis=0)
    return out.reshape(B, T, D).astype(np.float32)
